# revision 1
# baseline (speedup 1.0000x reference)
"""Bahdanau-attention Trainium2 kernel.

kernel(hidden, encoder_outputs, W, b, v) -> (context, attn_weights)
  hidden:          (1, 32, 256) f32
  encoder_outputs: (32, 8192, 256) f32
  W: (256, 512) f32   b: (256,) f32   v: (256,) f32
  context: (32, 256) f32    attn_weights: (32, 8192) f32

Data-parallel over batch: 8 NeuronCores x 4 batches each. All params
replicated. encoder_outputs is read from HBM exactly once per core.
"""
import sys
sys.path.insert(0, '/opt/trn_rl_repo')
import numpy as np
import concourse.bass as bass
import concourse.tile as tile
from concourse import bacc, mybir, bass_isa
from contextlib import ExitStack

F32 = mybir.dt.float32
F32R = mybir.dt.float32r
AF = mybir.ActivationFunctionType

H = 256          # hidden size
S = 8192         # sequence length
B = 32           # global batch
NCORES = 8
BLOC = B // NCORES          # batches per core = 4
TT = 1024                   # tokens per DMA tile
NT = S // TT                # DMA tiles per batch = 8
NSUB = S // 512             # 512-token subtiles per batch = 16


def _build():
    nc = bacc.Bacc("TRN2", target_bir_lowering=False, debug=False, num_devices=NCORES)

    enc_d  = nc.dram_tensor("enc",  [BLOC, S, H], F32, kind="ExternalInput").ap()
    weT_d  = nc.dram_tensor("weT",  [128, 2, H],  F32, kind="ExternalInput").ap()
    whT_d  = nc.dram_tensor("whT",  [128, 2, H],  F32, kind="ExternalInput").ap()
    hT_d   = nc.dram_tensor("hT",   [128, 2, BLOC], F32, kind="ExternalInput").ap()
    bl_d   = nc.dram_tensor("bl",   [128, 2],     F32, kind="ExternalInput").ap()
    vsel_d = nc.dram_tensor("vsel", [128, 2, 31], F32, kind="ExternalInput").ap()
    eye_d  = nc.dram_tensor("eye",  [128, 128],   F32, kind="ExternalInput").ap()

    attn_d = nc.dram_tensor("attn", [BLOC, S], F32, kind="ExternalOutput").ap()
    ctx_d  = nc.dram_tensor("ctx",  [BLOC, H], F32, kind="ExternalOutput").ap()

    enc_r = enc_d.bitcast(F32R)

    es = ExitStack()
    with tile.TileContext(nc) as tc:
        cst  = es.enter_context(tc.tile_pool(name="cst",  bufs=1))
        encp = es.enter_context(tc.tile_pool(name="encp", bufs=2 * NT))
        sbT  = es.enter_context(tc.tile_pool(name="sbT",  bufs=4))
        sbS  = es.enter_context(tc.tile_pool(name="sbS",  bufs=4))
        sbX  = es.enter_context(tc.tile_pool(name="sbX",  bufs=2))
        psA  = es.enter_context(tc.tile_pool(name="psA",  bufs=2, space="PSUM"))
        psB  = es.enter_context(tc.tile_pool(name="psB",  bufs=2, space="PSUM"))
        psL  = es.enter_context(tc.tile_pool(name="psL",  bufs=2, space="PSUM"))
        psC  = es.enter_context(tc.tile_pool(name="psC",  bufs=2, space="PSUM"))

        # constants
        weT = cst.tile([128, 2, H], F32R);   nc.sync.dma_start(weT[:], weT_d.bitcast(F32R))
        whT = cst.tile([128, 2, H], F32R);   nc.sync.dma_start(whT[:], whT_d.bitcast(F32R))
        hT  = cst.tile([128, 2, BLOC], F32R); nc.sync.dma_start(hT[:], hT_d.bitcast(F32R))
        bl  = cst.tile([128, 2], F32);       nc.sync.dma_start(bl[:], bl_d)
        vsel= cst.tile([128, 2, 31], F32R);  nc.sync.dma_start(vsel[:], vsel_d.bitcast(F32R))
        eye = cst.tile([128, 128], F32R);    nc.sync.dma_start(eye[:], eye_d.bitcast(F32R))

        # preh[p, oc, lb] = (Wh @ h_lb + b)[oc*128+p]
        preh = cst.tile([128, 2, BLOC], F32)
        for oc in range(2):
            pp = psB.tile([128, BLOC], F32, tag="pre")
            for kc in range(2):
                nc.tensor.matmul(pp[:], whT[:, kc, oc * 128:(oc + 1) * 128], hT[:, kc, :],
                                 start=(kc == 0), stop=(kc == 1))
            nc.scalar.activation(preh[:, oc, :], pp[:], AF.Identity,
                                 bias=bl[:, oc:oc + 1], scale=1.0)

        for lb in range(BLOC):
            logits_ps = psL.tile([16, 512], F32, tag="logits")
            ctx_ps = psC.tile([1, H], F32, tag="ctx")
            enc_tiles = []
            for st in range(NT):
                et = encp.tile([128, TT // 128, H], F32R, tag="enc")
                nc.sync.dma_start(
                    et[:], enc_r[lb, st * TT:(st + 1) * TT, :].rearrange("(j p) h -> p j h", p=128))
                enc_tiles.append(et)
                for sub in range(TT // 512):
                    t = st * 2 + sub          # 512-token subtile index 0..15
                    eTs = []
                    for kc in range(2):
                        eps = psA.tile([128, 512], F32R, tag="eT")
                        for j in range(4):
                            jj = sub * 4 + j
                            nc.tensor.transpose(eps[:, j * 128:(j + 1) * 128],
                                                et[:, jj, kc * 128:(kc + 1) * 128], eye[:])
                        esb = sbT.tile([128, 512], F32R, tag="eTsb")
                        nc.vector.tensor_copy(esb[:], eps[:])
                        eTs.append(esb)
                    for oc in range(2):
                        pps = psB.tile([128, 512], F32, tag="pre")
                        for kc in range(2):
                            nc.tensor.matmul(pps[:], weT[:, kc, oc * 128:(oc + 1) * 128],
                                             eTs[kc][:], start=(kc == 0), stop=(kc == 1))
                        scT = sbS.tile([128, 512], F32R, tag="scT")
                        nc.scalar.activation(scT[:], pps[:], AF.Tanh,
                                             bias=preh[:, oc, lb:lb + 1], scale=1.0)
                        nc.tensor.matmul(logits_ps[:], vsel[:, oc, 15 - t:31 - t], scT[:],
                                         start=(t == 0 and oc == 0),
                                         stop=(t == NSUB - 1 and oc == 1),
                                         skip_group_check=True)

            # ---- softmax over the 8192 logits of batch lb (layout [16, 512])
            mx   = sbX.tile([16, 1], F32, tag="mx")
            nc.vector.tensor_reduce(mx[:], logits_ps[:], axis=mybir.AxisListType.X,
                                    op=mybir.AluOpType.max)
            mxr  = sbX.tile([16, 1], F32, tag="mxr")
            nc.gpsimd.partition_all_reduce(mxr[:], mx[:], channels=16,
                                           reduce_op=bass_isa.ReduceOp.max)
            negm = sbX.tile([16, 1], F32, tag="negm")
            nc.vector.tensor_scalar_mul(negm[:], mxr[:], -1.0)
            ex   = sbX.tile([16, 512], F32, tag="ex")
            rows = sbX.tile([16, 1], F32, tag="rows")
            nc.scalar.activation(ex[:], logits_ps[:], AF.Exp,
                                 bias=negm[:, 0:1], scale=1.0, accum_out=rows[:])
            Z    = sbX.tile([16, 1], F32, tag="Z")
            nc.gpsimd.partition_all_reduce(Z[:], rows[:], channels=16,
                                           reduce_op=bass_isa.ReduceOp.add)
            invZ = sbX.tile([16, 1], F32, tag="invZ")
            nc.vector.reciprocal(invZ[:], Z[:])
            w_f  = sbX.tile([16, 512], F32, tag="wf")
            nc.vector.tensor_scalar_mul(w_f[:], ex[:], invZ[:, 0:1])
            w_r  = sbX.tile([16, 512], F32R, tag="wr")
            nc.vector.tensor_scalar_mul(w_r[:], ex[:], invZ[:, 0:1])
            nc.sync.dma_start(attn_d[lb].rearrange("(t n) -> t n", n=512), w_f[:])

            # ---- transpose w into [128, (4 chunks x 16 tiles)] for ctx lhsT
            wT = sbX.tile([128, 64], F32R, tag="wT")
            for c in range(4):
                wps = psA.tile([128, 16], F32R, tag="eT")
                nc.tensor.transpose(wps[:], w_r[:, c * 128:(c + 1) * 128], eye[:16, :16])
                nc.vector.tensor_copy(wT[:, c * 16:(c + 1) * 16], wps[:])

            # ---- pass 2: context accumulation over SBUF-resident enc
            for st in range(NT):
                for jj in range(TT // 128):
                    t = (st * TT + jj * 128) // 512
                    c = jj % 4
                    nc.tensor.matmul(ctx_ps[:], wT[:, c * 16 + t:c * 16 + t + 1],
                                     enc_tiles[st][:, jj, :],
                                     start=(st == 0 and jj == 0),
                                     stop=(st == NT - 1 and jj == TT // 128 - 1),
                                     skip_group_check=True)
            csb = sbX.tile([1, H], F32, tag="csb")
            nc.vector.tensor_copy(csb[:], ctx_ps[:])
            nc.sync.dma_start(ctx_d[lb:lb + 1, :], csb[:])
        es.close()

    nc.compile()
    return nc


_NC = None


def _prep_inputs(hidden, encoder_outputs, W, b, v):
    hidden = np.ascontiguousarray(hidden, dtype=np.float32)
    enc = np.ascontiguousarray(encoder_outputs, dtype=np.float32)
    W = np.asarray(W, dtype=np.float32)
    b = np.asarray(b, dtype=np.float32)
    v = np.asarray(v, dtype=np.float32)

    Wh, We = W[:, :H], W[:, H:]
    weT = np.ascontiguousarray(We.T.reshape(2, 128, H).transpose(1, 0, 2))
    whT = np.ascontiguousarray(Wh.T.reshape(2, 128, H).transpose(1, 0, 2))
    h_all = hidden[0]                                    # (32, 256)
    hT_full = np.ascontiguousarray(h_all.T.reshape(2, 128, B).transpose(1, 0, 2))  # (128,2,32)
    bl = np.ascontiguousarray(b.reshape(2, 128).T)       # (128, 2)
    vsel = np.zeros((128, 2, 31), dtype=np.float32)
    vsel[:, :, 15] = v.reshape(2, 128).T
    eye = np.eye(128, dtype=np.float32)

    in_maps = []
    for cid in range(NCORES):
        sl = slice(cid * BLOC, (cid + 1) * BLOC)
        in_maps.append({
            "enc": enc[sl],
            "weT": weT, "whT": whT,
            "hT": np.ascontiguousarray(hT_full[:, :, sl]),
            "bl": bl, "vsel": vsel, "eye": eye,
        })
    return in_maps


def get_nc():
    global _NC
    if _NC is None:
        _NC = _build()
    return _NC


def kernel(hidden, encoder_outputs, W, b, v):
    from concourse.bass_utils import run_bass_kernel_spmd
    nc = get_nc()
    in_maps = _prep_inputs(hidden, encoder_outputs, W, b, v)
    res = run_bass_kernel_spmd(nc, in_maps, core_ids=list(range(NCORES)))
    ctx = np.concatenate([r["ctx"] for r in res.results], axis=0)
    attn = np.concatenate([r["attn"] for r in res.results], axis=0)
    return (ctx, attn)


# revision 11
# speedup vs baseline: 334.6486x; 334.6486x over previous
"""Bahdanau-attention Trainium2 kernel.

kernel(hidden, encoder_outputs, W, b, v) -> (context, attn_weights)
  hidden:          (1, 32, 256) f32
  encoder_outputs: (32, 8192, 256) f32
  W: (256, 512) f32   b: (256,) f32   v: (256,) f32
  context: (32, 256) f32    attn_weights: (32, 8192) f32

Data-parallel over batch: 8 NeuronCores x 4 batches each. All params
replicated. encoder_outputs is read from HBM exactly once per core, kept
SBUF-resident per batch; logits/softmax/context computed on-chip.
Software-pipelined emission: softmax + context-pass of batch lb-1 are
interleaved into pass-1 of batch lb.
"""
import sys
sys.path.insert(0, '/opt/trn_rl_repo')
import numpy as np
import concourse.bass as bass
import concourse.tile as tile
from concourse import bacc, mybir, bass_isa
from contextlib import ExitStack

F32 = mybir.dt.float32
F32R = mybir.dt.float32r
AF = mybir.ActivationFunctionType

H = 256          # hidden size
S = 8192         # sequence length
B = 32           # global batch
NCORES = 8
BLOC = B // NCORES          # batches per core = 4
TT = 1024                   # tokens per DMA tile
NT = S // TT                # DMA tiles per batch = 8
NSUB = S // 512             # 512-token subtiles per batch = 16


def _build(krep=1):
    nc = bacc.Bacc("TRN2", target_bir_lowering=False, debug=False, num_devices=NCORES)

    enc_d  = nc.dram_tensor("enc",  [BLOC, S, H], F32, kind="ExternalInput").ap()
    eye_d  = nc.dram_tensor("eye",  [128, 128],   F32, kind="ExternalInput").ap()
    # packed fp32r params: weT(512) | whT(512) | hT(2*BLOC) | vsel(62)
    PK = 512 + 512 + 2 * BLOC + 62
    pk_d   = nc.dram_tensor("pk", [128, PK], F32, kind="ExternalInput").ap()
    bl_d   = nc.dram_tensor("bl", [128, 2],  F32, kind="ExternalInput").ap()

    attn_d = nc.dram_tensor("attn", [BLOC, S], F32, kind="ExternalOutput").ap()
    ctx_d  = nc.dram_tensor("ctx",  [BLOC, H], F32, kind="ExternalOutput").ap()

    enc_r = enc_d.bitcast(F32R)

    es = ExitStack()
    with tile.TileContext(nc) as tc:
        cst  = es.enter_context(tc.tile_pool(name="cst",  bufs=1))
        encp = es.enter_context(tc.tile_pool(name="encp", bufs=2 * NT))
        sbT  = es.enter_context(tc.tile_pool(name="sbT",  bufs=4))
        sbS  = es.enter_context(tc.tile_pool(name="sbS",  bufs=6))
        sbX  = es.enter_context(tc.tile_pool(name="sbX",  bufs=2))
        psA  = es.enter_context(tc.tile_pool(name="psA",  bufs=2, space="PSUM"))
        psB  = es.enter_context(tc.tile_pool(name="psB",  bufs=2, space="PSUM"))
        psL  = es.enter_context(tc.tile_pool(name="psL",  bufs=2, space="PSUM"))
        psC  = es.enter_context(tc.tile_pool(name="psC",  bufs=2, space="PSUM"))

        # constants: eye first (gates the first transposes), then the packed rest
        eye = cst.tile([128, 128], F32R);  nc.sync.dma_start(eye[:], eye_d.bitcast(F32R))
        pk  = cst.tile([128, PK], F32R);   nc.sync.dma_start(pk[:], pk_d.bitcast(F32R))
        bl  = cst.tile([128, 2], F32);     nc.sync.dma_start(bl[:], bl_d)
        weT  = pk[:, 0:512].rearrange("p (kc h) -> p kc h", h=H)
        whT  = pk[:, 512:1024].rearrange("p (kc h) -> p kc h", h=H)
        hT   = pk[:, 1024:1024 + 2 * BLOC].rearrange("p (kc l) -> p kc l", l=BLOC)
        vsel = pk[:, 1024 + 2 * BLOC:1024 + 2 * BLOC + 62].rearrange(
            "p (kc c) -> p kc c", c=31)

        # preh[p, oc, lb] = (Wh @ h_lb + b)[oc*128+p]  (emitted lazily below so
        # the PE's first instructions are the first tile's transposes)
        preh = cst.tile([128, 2, BLOC], F32)

        def emit_preh():
            for oc in range(2):
                pp = psB.tile([128, BLOC], F32, tag="pre", name="pp_preh")
                for kc in range(2):
                    nc.tensor.matmul(pp[:], whT[:, kc, oc * 128:(oc + 1) * 128],
                                     hT[:, kc, :], start=(kc == 0), stop=(kc == 1))
                nc.scalar.activation(preh[:, oc, :], pp[:], AF.Identity,
                                     bias=bl[:, oc:oc + 1], scale=1.0)

        # ---------- per-batch step emitters ----------
        def softmax_part1(st_):
            """logits -> global max -> -max  (DVE + gpsimd, no ACT)"""
            logits_ps = st_["logits_ps"]
            mx = sbX.tile([16, 1], F32, tag="mx")
            nc.vector.tensor_reduce(mx[:], logits_ps[:], axis=mybir.AxisListType.X,
                                    op=mybir.AluOpType.max)
            mxr = sbX.tile([16, 1], F32, tag="mxr")
            nc.gpsimd.partition_all_reduce(mxr[:], mx[:], channels=16,
                                           reduce_op=bass_isa.ReduceOp.max)
            negm = sbX.tile([16, 1], F32, tag="negm")
            nc.vector.tensor_scalar_mul(negm[:], mxr[:], -1.0)
            st_["negm"] = negm

        def softmax_part2(st_, lb):
            """exp/Z/weights + attn output + wT transposes"""
            logits_ps, negm = st_["logits_ps"], st_["negm"]
            ex = sbX.tile([16, 512], F32, tag="ex")
            rows = sbX.tile([16, 1], F32, tag="rows")
            nc.scalar.activation(ex[:], logits_ps[:], AF.Exp,
                                 bias=negm[:, 0:1], scale=1.0, accum_out=rows[:])
            Z = sbX.tile([16, 1], F32, tag="Z")
            nc.gpsimd.partition_all_reduce(Z[:], rows[:], channels=16,
                                           reduce_op=bass_isa.ReduceOp.add)
            invZ = sbX.tile([16, 1], F32, tag="invZ")
            nc.vector.reciprocal(invZ[:], Z[:])
            w_f = sbX.tile([16, 512], F32, tag="wf")
            nc.vector.tensor_scalar_mul(w_f[:], ex[:], invZ[:, 0:1])
            w_r = sbX.tile([16, 512], F32R, tag="wr")
            nc.vector.tensor_scalar_mul(w_r[:], ex[:], invZ[:, 0:1])
            nc.sync.dma_start(attn_d[lb].rearrange("(t n) -> t n", n=512), w_f[:])
            wT = sbX.tile([128, 64], F32R, tag="wT")
            for c in range(4):
                wps = psA.tile([128, 16], F32R, tag="eT")
                nc.tensor.transpose(wps[:], w_r[:, c * 128:(c + 1) * 128], eye[:16, :16])
                nc.vector.tensor_copy(wT[:, c * 16:(c + 1) * 16], wps[:])
            st_["wT"] = wT

        def ctx_chunk(st_, lb, blocks):
            """emit ctx matmuls for block indices in `blocks` (global 0..63)"""
            if "ctx_ps" not in st_:
                st_["ctx_ps"] = psC.tile([1, H], F32, tag="ctx", name="ctx_ps")
            ctx_ps, wT = st_["ctx_ps"], st_["wT"]
            for gi in blocks:
                et, jj = st_["blk"][gi]
                t, c = gi // 4, gi % 4
                nc.tensor.matmul(ctx_ps[:], wT[:, c * 16 + t:c * 16 + t + 1],
                                 et[:, jj, :],
                                 start=(gi == 0), stop=(gi == NT * (TT // 128) - 1),
                                 skip_group_check=True)
            if blocks[-1] == NT * (TT // 128) - 1:
                csb = sbX.tile([1, H], F32, tag="csb")
                nc.vector.tensor_copy(csb[:], ctx_ps[:])
                nc.sync.dma_start(ctx_d[lb:lb + 1, :], csb[:])

        NBLK = NT * (TT // 128)          # 64 ctx blocks per batch

        # ---------- global software-pipelined emission ----------
        # stage A (subtile t):   transposes -> psA, DVE copies -> eTs
        # stage B (subtile t+1): WeMM + tanh for t
        # stage C (subtile t+2): logits matmuls for t
        # epilogue of batch lb runs during subtiles 2..15 of batch lb+1
        pend_we = None    # (lb, t, eTs)
        pend_vmm = None   # (lb, t, scts, logits_ps)
        states = {}

        def stage_b(lb, t, eTs):
            st_ = states[lb]
            scts = []
            for oc in range(2):
                pps = psB.tile([128, 512], F32, tag="pre")
                for kc in range(2):
                    nc.tensor.matmul(pps[:], weT[:, kc, oc * 128:(oc + 1) * 128],
                                     eTs[kc][:], start=(kc == 0), stop=(kc == 1))
                scT = sbS.tile([128, 512], F32R, tag="scT")
                nc.scalar.activation(scT[:], pps[:], AF.Tanh,
                                     bias=preh[:, oc, lb:lb + 1], scale=1.0)
                scts.append(scT)
            return scts

        def stage_c(lb, t, scts):
            st_ = states[lb]
            for oc in range(2):
                nc.tensor.matmul(st_["logits_ps"][:],
                                 vsel[:, oc, 15 - t:31 - t], scts[oc][:],
                                 start=(t == 0 and oc == 0),
                                 stop=(t == NSUB - 1 and oc == 1),
                                 skip_group_check=True)

        for rep in range(krep):
            for lb in range(BLOC):
                st_ = {"blk": []}
                st_["logits_ps"] = psL.tile([16, 512], F32, tag="logits", name="logits_ps")
                states[lb] = st_
                sched = {}
                if lb > 0:
                    p = lb - 1
                    sched[2] = [lambda p=p: softmax_part1(states[p])]
                    sched[3] = [lambda p=p: softmax_part2(states[p], p)]
                    n_steps = NSUB - 4   # subtiles 4..15
                    per = (NBLK + n_steps - 1) // n_steps
                    for i in range(n_steps):
                        blk = list(range(NBLK))[i * per:(i + 1) * per]
                        if blk:
                            sched.setdefault(4 + i, []).append(
                                lambda p=p, b_=blk: ctx_chunk(states[p], p, b_))

                for stt in range(NT):
                    if rep == 0 and lb == 0 and stt == 0:
                        # split the very first tile's DMA so the pipeline
                        # starts without waiting for a full 1 MiB transfer
                        ets = []
                        for q in range(4):
                            etq = encp.tile([128, TT // 512, H], F32R,
                                            tag="enc_head", name=f"enc_head{q}", bufs=4)
                            nc.sync.dma_start(
                                etq[:], enc_r[lb, q * (TT // 4):(q + 1) * (TT // 4), :]
                                .rearrange("(j p) h -> p j h", p=128))
                            ets.append(etq)
                        for jj in range(TT // 128):
                            st_["blk"].append((ets[jj // 2], jj % 2))
                    else:
                        et = encp.tile([128, TT // 128, H], F32R, tag="enc")
                        nc.sync.dma_start(
                            et[:], enc_r[lb, stt * TT:(stt + 1) * TT, :]
                            .rearrange("(j p) h -> p j h", p=128))
                        for jj in range(TT // 128):
                            st_["blk"].append((et, jj))
                    for sub in range(TT // 512):
                        t = stt * 2 + sub
                        # stage A for (lb, t)
                        eTs = []
                        for kc in range(2):
                            eps = psA.tile([128, 512], F32R, tag="eT")
                            for j in range(4):
                                gi = t * 4 + j
                                bet, bjj = st_["blk"][gi]
                                nc.tensor.transpose(eps[:, j * 128:(j + 1) * 128],
                                                    bet[:, bjj, kc * 128:(kc + 1) * 128],
                                                    eye[:])
                            esb = sbT.tile([128, 512], F32R, tag="eTsb")
                            nc.vector.tensor_copy(esb[:], eps[:])
                            eTs.append(esb)
                        # stage B for previous subtile
                        if pend_we is not None:
                            plb, pt, peTs = pend_we
                            pscts = stage_b(plb, pt, peTs)
                            if pend_vmm is not None:
                                vlb, vt, vscts = pend_vmm
                                stage_c(vlb, vt, vscts)
                            pend_vmm = (plb, pt, pscts)
                        if rep == 0 and lb == 0 and t == 0:
                            emit_preh()
                        pend_we = (lb, t, eTs)
                        for fn in sched.pop(t, []):
                            fn()
            # pipeline drain for this rep
            plb, pt, peTs = pend_we
            pscts = stage_b(plb, pt, peTs)
            if pend_vmm is not None:
                vlb, vt, vscts = pend_vmm
                stage_c(vlb, vt, vscts)
            stage_c(plb, pt, pscts)
            pend_we = pend_vmm = None
            # epilogue of the final batch
            softmax_part1(states[BLOC - 1])
            softmax_part2(states[BLOC - 1], BLOC - 1)
            ctx_chunk(states[BLOC - 1], BLOC - 1, list(range(NBLK)))
        es.close()

    nc.compile()
    return nc


_NC = None


def _prep_inputs(hidden, encoder_outputs, W, b, v):
    hidden = np.ascontiguousarray(hidden, dtype=np.float32)
    enc = np.ascontiguousarray(encoder_outputs, dtype=np.float32)
    W = np.asarray(W, dtype=np.float32)
    b = np.asarray(b, dtype=np.float32)
    v = np.asarray(v, dtype=np.float32)

    Wh, We = W[:, :H], W[:, H:]
    weT = We.T.reshape(2, 128, H).transpose(1, 0, 2).reshape(128, 512)
    whT = Wh.T.reshape(2, 128, H).transpose(1, 0, 2).reshape(128, 512)
    h_all = hidden[0]                                    # (32, 256)
    hT_full = h_all.T.reshape(2, 128, B).transpose(1, 0, 2)   # (128, 2, 32)
    bl = b.reshape(2, 128).T                             # (128, 2)
    vsel = np.zeros((128, 2, 31), dtype=np.float32)
    vsel[:, :, 15] = v.reshape(2, 128).T
    eye = np.eye(128, dtype=np.float32)

    in_maps = []
    for cid in range(NCORES):
        sl = slice(cid * BLOC, (cid + 1) * BLOC)
        pk = np.concatenate([
            weT, whT,
            hT_full[:, :, sl].reshape(128, 2 * BLOC),
            vsel.reshape(128, 62),
        ], axis=1).astype(np.float32)
        in_maps.append({"enc": enc[sl], "eye": eye,
                        "pk": np.ascontiguousarray(pk),
                        "bl": np.ascontiguousarray(bl).astype(np.float32)})
    return in_maps


def get_nc():
    global _NC
    if _NC is None:
        _NC = _build()
    return _NC


def kernel(hidden, encoder_outputs, W, b, v):
    from concourse.bass_utils import run_bass_kernel_spmd
    nc = get_nc()
    in_maps = _prep_inputs(hidden, encoder_outputs, W, b, v)
    res = run_bass_kernel_spmd(nc, in_maps, core_ids=list(range(NCORES)))
    ctx = np.concatenate([r["ctx"] for r in res.results], axis=0)
    attn = np.concatenate([r["attn"] for r in res.results], axis=0)
    return (ctx, attn)
